# revision 30
# baseline (speedup 1.0000x reference)
"""Trainium2 Bass kernel for nn_BrainRegion (liquid-gated recurrent cell).

Computes, for full inputs (B=8192, IN=H=2048):
    xin  = concat([x_t, state], -1)
    cand = tanh(xin @ Wc + state @ Uc + bc)
    gate = sigmoid(xin @ Wg + state @ Ug + bg)
    alpha = exp(-1/exp(log_step))
    h    = alpha * state + (1 - alpha) * gate * cand
    out  = layernorm(h) * gamma + beta

Strategy: data-parallel over batch across 8 NeuronCores (1024 rows/core),
weights replicated.  Algebraic fold: xin@Wc + state@Uc == x_t@Wc[:IN] +
state@(Wc[IN:] + Uc), which removes one third of the FLOPs.  Mixed
precision: the sigmoid (gate) path runs entirely in fp8 e4m3 with
DoubleRow perf mode (2x tensor throughput; sigmoid' <= 0.25 compresses
the quantization error).  The tanh (cand) path splits: its x_t part is
fp8 DoubleRow (reusing the gate's quantized activations), while its
state part stays bf16 -- the folded state weights (Wc[IN:]+Uc) carry 3x
the variance of Wc[:IN], so they dominate the quantization error and
are kept in high precision.  The bf16 weights are pre-scaled by
4096 == SX*SW (an exact power of two) so both parts accumulate into a
single PSUM stream and one epilogue descale serves the whole path.
PSUM accumulates in fp32; the elementwise epilogue + layernorm run
on-device in fp32; h/state/output in bf16.
"""

import sys

if "/opt/trn_rl_repo" not in sys.path:
    sys.path.insert(0, "/opt/trn_rl_repo")

import numpy as np
import ml_dtypes

B, IN, H = 8192, 2048, 2048
NCORES = 8
BC = B // NCORES      # rows per core (1024)
P = 128               # partitions
G = BC // P           # batch groups per core (8)
NJ = 8                # H slices for cand/epilogue
NSL = H // NJ         # slice width (256)
NJG = 4               # H slices for the fp8 gate matmuls
NGL = H // NJG        # gate slice width (512)
KT = H // P           # k-tiles per matrix (16)
KP = KT // 2          # fp8 DoubleRow k-pairs (8)
FSK = 2               # cand-s k-tiles computed in fp8 (error budget cap)
KTB = KT - FSK        # cand-s k-tiles in bf16 (14)
EPS = 1e-5

bf16 = ml_dtypes.bfloat16
e4m3 = ml_dtypes.float8_e4m3
SX = 16.0             # gate activation quant scale
SW = 256.0            # gate weight quant scale
DESCALE = 1.0 / (SX * SW)

# Set by test.py to collect a hardware profile.
TRACE = False
LAST_RESULTS = None

_compiled = {}


ALPHA0 = float(np.exp(-1.0))  # alpha when log_step == 0


def _build(flags):
    """Trace + compile the SPMD device program. flags = (has_bc, has_bg,
    has_gamma, has_beta, has_logstep) selects optional elementwise
    passes."""
    from contextlib import ExitStack

    import concourse.bass as bass
    import concourse.tile as tile
    from concourse import bacc, mybir

    has_bc, has_bg, has_gamma, has_beta, has_logstep = flags
    f32 = mybir.dt.float32
    bft = mybir.dt.bfloat16
    f8 = mybir.dt.float8e4
    DR = mybir.MatmulPerfMode.DoubleRow
    AF = mybir.ActivationFunctionType
    OP = mybir.AluOpType

    nc = bacc.Bacc("TRN2", target_bir_lowering=False, debug=False,
                   num_devices=NCORES)

    # DRAM I/O. Activation tensors are pre-arranged on host so every DMA
    # below is contiguous:
    #   sb4:     [G, P, KTB, P]  bf16, [g,p,k,m] = s[g*128+m, (k+FSK)*128+p]
    #   xq4/sq4: [G, P, KT, P]   fp8 (x*SX), [g,p,k,m] = x[g*128+m, k*128+p]
    #   wcx:     [NJG, P, KT, NGL] fp8 (W*SW), [j,p,k,n] = W[k*128+p, j*NGL+n]
    #   wcs8:    [NJG, P, FSK, NGL] fp8 (W*SW), first FSK k-tiles of Wcs
    #   wcs:     [NJG, P, KTB, NGL] bf16 (W*SX*SW), remaining k-tiles
    #   wg*:     [NJG, P, KT, NGL] fp8 (W*SW), same arrangement
    sb4 = nc.dram_tensor("sb4", [G, P, KTB, P], bft,
                         kind="ExternalInput").ap()
    xq4 = nc.dram_tensor("xq4", [G, P, KT, P], f8, kind="ExternalInput").ap()
    sq4 = nc.dram_tensor("sq4", [G, P, KT, P], f8, kind="ExternalInput").ap()
    stb = nc.dram_tensor("stb", [BC, H], bft, kind="ExternalInput").ap()
    wcx = nc.dram_tensor("wcx", [NJG, P, KT, NGL], f8,
                         kind="ExternalInput").ap()
    wcs8 = nc.dram_tensor("wcs8", [NJG, P, FSK, NGL], f8,
                          kind="ExternalInput").ap()
    wcs = nc.dram_tensor("wcs", [NJG, P, KTB, NGL], bft,
                         kind="ExternalInput").ap()
    wgx = nc.dram_tensor("wgx", [NJG, P, KT, NGL], f8,
                         kind="ExternalInput").ap()
    wgs = nc.dram_tensor("wgs", [NJG, P, KT, NGL], f8,
                         kind="ExternalInput").ap()
    if has_logstep:
        logb = nc.dram_tensor("logb", [P, H], f32,
                              kind="ExternalInput").ap()
    vecs = {}
    for name, used in (("bcb", has_bc), ("bgb", has_bg),
                       ("gammab", has_gamma), ("betab", has_beta)):
        if used:
            vecs[name] = nc.dram_tensor(name, [P, H], f32,
                                        kind="ExternalInput").ap()
    out = nc.dram_tensor("out", [BC, H], bft, kind="ExternalOutput").ap()

    with tile.TileContext(nc) as tc, ExitStack() as ctx:
        singles = ctx.enter_context(tc.tile_pool(name="singles", bufs=1))
        gactp = ctx.enter_context(tc.tile_pool(name="gactp", bufs=1))
        cactp = ctx.enter_context(tc.tile_pool(name="cactp", bufs=2))
        wcp = ctx.enter_context(tc.tile_pool(name="wcp", bufs=2))
        wgp = ctx.enter_context(tc.tile_pool(name="wgp", bufs=2))
        psgp = ctx.enter_context(tc.tile_pool(name="psgp", bufs=3,
                                              space="PSUM"))
        pscp = ctx.enter_context(tc.tile_pool(name="pscp", bufs=3,
                                              space="PSUM"))
        epp = ctx.enter_context(tc.tile_pool(name="epp", bufs=2))
        stp = ctx.enter_context(tc.tile_pool(name="stp", bufs=3))
        hp = ctx.enter_context(tc.tile_pool(name="hp", bufs=1))
        statp = ctx.enter_context(tc.tile_pool(name="statp", bufs=1))
        normp = ctx.enter_context(tc.tile_pool(name="normp", bufs=4))
        outp = ctx.enter_context(tc.tile_pool(name="outp", bufs=2))

        # ---- gate fp8 activations: resident for the whole kernel.
        # DMA'd lazily inside the first jg sweep so the first weight
        # slices aren't stuck behind 4 MB of activation DMA.
        xq_t = [gactp.tile([P, KT, P], f8, name=f"xq_g{g}", tag=f"xq{g}")
                for g in range(G)]
        sq_t = [gactp.tile([P, KT, P], f8, name=f"sq_g{g}", tag=f"sq{g}")
                for g in range(G)]

        # ---- constants: alpha = exp(-exp(-log_step)), broadcast [P, H].
        # When log_step == 0 (has_logstep False) alpha is the compile-time
        # scalar ALPHA0 and no tile is needed.
        if has_logstep:
            alpha_t = singles.tile([P, H], f32, name="alpha_t")
            nc.sync.dma_start(out=alpha_t[:], in_=logb[:])
            nc.scalar.activation(alpha_t[:], alpha_t[:], AF.Exp, scale=-1.0)
            nc.scalar.activation(alpha_t[:], alpha_t[:], AF.Exp, scale=-1.0)
        eps_t = singles.tile([P, 1], f32, name="eps_t")
        nc.vector.memset(eps_t[:], EPS)
        vt = {}
        for name in vecs:
            vt[name] = singles.tile([P, H], f32, name=name + "_t")
            nc.sync.dma_start(out=vt[name][:], in_=vecs[name][:])

        # ---- per-group h accumulator (bf16) and layernorm stats ----
        h_t = [hp.tile([P, H], bft, name=f"h_g{g}", tag=f"h{g}")
               for g in range(G)]
        stats_t = [statp.tile([P, NJ, 6], f32, name=f"stats_g{g}",
                              tag=f"st{g}")
                   for g in range(G)]

        # No PE warm-up: the first-iteration weight DMA is HBM-bound, so
        # the real matmul stream (started as soon as the first wgx chunk
        # lands) absorbs the tensor-clock ramp while the rest of the
        # weights arrive -- dummy warm-up matmuls only delay it.

        # ---- main loops: jg = gate H slice (2 cand slices), g = batch ----
        prefetched = {}
        for jg in range(NJG):
            wgx_t = wgp.tile([P, KT, NGL], f8, name=f"wgx_{jg}", tag="wgx")
            wgs_t = wgp.tile([P, KT, NGL], f8, name=f"wgs_{jg}", tag="wgs")
            wcx_t = wcp.tile([P, KT, NGL], f8, name=f"wcx_{jg}", tag="wcx")
            wcs8_t = wcp.tile([P, FSK, NGL], f8, name=f"wcs8_{jg}",
                              tag="wcs8")
            wcs_t = wcp.tile([P, KTB, NGL], bft, name=f"wcs_{jg}", tag="wcs")
            if jg == 0:
                # First iteration: every matmul stream is on the critical
                # path.  Chunk the weight tiles k-wise so the 14 streams
                # land on distinct DMA queues (a queue moves ~8KB/us; a
                # whole tile on one queue costs 16us); issue in consumption
                # order: gate-x, gate-s, cand-x, cand-s.
                KH = KT // 2
                # wcs chunk 0 first: the cand-s bf16 stream consumes
                # k-tiles in order, and range-level deps let it start on
                # the first 4 k-tiles alone
                nc.sync.dma_start(out=wcs_t[:, 0:4, :],
                                  in_=wcs[jg, :, 0:4, :])
                nc.sync.dma_start(out=wgx_t[:, :KH, :],
                                  in_=wgx[jg, :, :KH, :])
                nc.sync.dma_start(out=xq_t[0][:], in_=xq4[0])
                nc.sync.dma_start(out=sq_t[0][:], in_=sq4[0])
                nc.sync.dma_start(out=wgx_t[:, KH:, :],
                                  in_=wgx[jg, :, KH:, :])
                for ck, ce in ((4, 8), (8, 11), (11, KTB)):
                    nc.sync.dma_start(out=wcs_t[:, ck:ce, :],
                                      in_=wcs[jg, :, ck:ce, :])
                for c in range(2):
                    nc.sync.dma_start(out=wgs_t[:, c * KH:(c + 1) * KH, :],
                                      in_=wgs[jg, :, c * KH:(c + 1) * KH, :])
                for c in range(2):
                    nc.sync.dma_start(out=wcx_t[:, c * KH:(c + 1) * KH, :],
                                      in_=wcx[jg, :, c * KH:(c + 1) * KH, :])
                nc.sync.dma_start(out=wcs8_t[:], in_=wcs8[jg])
                sb0 = cactp.tile([P, KTB, P], bft, name="sb_0_0", tag="sb")
                KBH = KTB // 2
                nc.sync.dma_start(out=sb0[:, :KBH, :],
                                  in_=sb4[0, :, :KBH, :])
                nc.sync.dma_start(out=sb0[:, KBH:, :],
                                  in_=sb4[0, :, KBH:, :])
                prefetched[(0, 0)] = sb0
            else:
                nc.sync.dma_start(out=wgx_t[:], in_=wgx[jg])
                nc.sync.dma_start(out=wgs_t[:], in_=wgs[jg])
                nc.sync.dma_start(out=wcx_t[:], in_=wcx[jg])
                nc.sync.dma_start(out=wcs8_t[:], in_=wcs8[jg])
                nc.sync.dma_start(out=wcs_t[:], in_=wcs[jg])

            # jg=0 runs a two-phase schedule: the fp8 gate/cand-x streams
            # for g0 AND g1 are issued before either group's bf16 cand-s,
            # so ~11us of DR matmuls cover the HBM-bound arrival of the
            # 1.75MB wcs tile instead of stalling on it.
            if jg == 0:
                schedule = [(0, "mm"), (1, "mm"), (0, "ep"), (1, "ep")]
                schedule += [(g, "full") for g in range(2, G)]
            else:
                schedule = [(g, "full") for g in range(G)]
            pend = {}
            for g, phase in schedule:
                if jg == 0 and g > 1 and phase == "full":
                    # delay these prefetches past the first-iteration
                    # critical DMA window -- all queues share ~358GB/s of
                    # HBM, so the 3MB would starve the wgs/wcx/wcs
                    # arrivals the first matmul streams wait on
                    with tc.tile_wait_until(0.014):
                        nc.sync.dma_start(out=xq_t[g][:], in_=xq4[g])
                        nc.sync.dma_start(out=sq_t[g][:], in_=sq4[g])
                elif jg == 0 and g == 1 and phase == "mm":
                    nc.sync.dma_start(out=xq_t[g][:], in_=xq4[g])
                    nc.sync.dma_start(out=sq_t[g][:], in_=sq4[g])

                if phase in ("mm", "full"):
                    # gate: fp8 DoubleRow matmuls, 512-wide moving stream
                    pg = psgp.tile([P, NGL], f32, name=f"pg_{jg}_{g}",
                                   tag="pg")
                    for kp in range(KP):
                        ks = slice(2 * kp, 2 * kp + 2)
                        nc.tensor.matmul(pg[:], xq_t[g][:, ks, :],
                                         wgx_t[:, ks, :],
                                         start=(kp == 0), stop=False,
                                         perf_mode=DR)
                    for kp in range(KP):
                        ks = slice(2 * kp, 2 * kp + 2)
                        nc.tensor.matmul(pg[:], sq_t[g][:, ks, :],
                                         wgs_t[:, ks, :],
                                         start=False, stop=(kp == KP - 1),
                                         perf_mode=DR)

                    # cand: x part fp8 DoubleRow, first FSK state k-tiles
                    # fp8 DoubleRow (via the resident gate activations);
                    # the bf16 remainder follows in the "ep" phase
                    pc = pscp.tile([P, NGL], f32, name=f"pc_{jg}_{g}",
                                   tag="pc")
                    for kp in range(KP):
                        ks = slice(2 * kp, 2 * kp + 2)
                        nc.tensor.matmul(pc[:], xq_t[g][:, ks, :],
                                         wcx_t[:, ks, :],
                                         start=(kp == 0), stop=False,
                                         perf_mode=DR)
                    for kp in range(FSK // 2):
                        ks = slice(2 * kp, 2 * kp + 2)
                        nc.tensor.matmul(pc[:], sq_t[g][:, ks, :],
                                         wcs8_t[:, ks, :],
                                         start=False, stop=False,
                                         perf_mode=DR)
                    if phase == "mm":
                        pend[g] = (pg, pc)
                        continue
                else:
                    pg, pc = pend.pop(g)

                # stream the cand bf16 state activations for this (jg, g),
                # then the bf16 remainder of the cand stream (weights
                # pre-scaled by SX*SW so all parts share one PSUM scale)
                if (jg, g) in prefetched:
                    sb_t = prefetched.pop((jg, g))
                else:
                    sb_t = cactp.tile([P, KTB, P], bft, name=f"sb_{jg}_{g}",
                                      tag="sb")
                    nc.sync.dma_start(out=sb_t[:], in_=sb4[g])
                for k in range(KTB):
                    nc.tensor.matmul(pc[:], sb_t[:, k, :],
                                     wcs_t[:, k, :],
                                     start=False, stop=(k == KTB - 1))

                for j in (2 * jg, 2 * jg + 1):
                    jsl = slice(j * NSL, (j + 1) * NSL)
                    off = (j - 2 * jg) * NSL
                    pcs = pc[:, off:off + NSL]
                    pgs = pg[:, off:off + NSL]

                    # epilogue for this (g, j) slice
                    sc = epp.tile([P, NSL], f32, name=f"sc_{j}_{g}",
                                  tag="sc")
                    sg = epp.tile([P, NSL], f32, name=f"sg_{j}_{g}",
                                  tag="sg")
                    if has_bc:
                        nc.vector.scalar_tensor_tensor(
                            sc[:], pcs, DESCALE, vt["bcb"][:, jsl],
                            op0=OP.mult, op1=OP.add)
                        nc.scalar.activation(sc[:], sc[:], AF.Tanh)
                    else:
                        nc.scalar.activation(sc[:], pcs, AF.Tanh,
                                             scale=DESCALE)
                    if has_bg:
                        nc.vector.scalar_tensor_tensor(
                            sg[:], pgs, DESCALE, vt["bgb"][:, jsl],
                            op0=OP.mult, op1=OP.add)
                        nc.scalar.activation(sg[:], sg[:], AF.Sigmoid)
                    else:
                        nc.scalar.activation(sg[:], pgs, AF.Sigmoid,
                                             scale=DESCALE)

                    st_sl = stp.tile([P, NSL], bft, name=f"stsl_{j}_{g}",
                                     tag="stsl")
                    nc.sync.dma_start(
                        out=st_sl[:],
                        in_=stb[g * P:(g + 1) * P, jsl])

                    # h = gc + alpha*(state - gc), with gc = gate*cand
                    t2 = epp.tile([P, NSL], f32, name=f"t2_{j}_{g}",
                                  tag="t2")
                    nc.vector.tensor_mul(t2[:], sc[:], sg[:])   # gate*cand
                    t3 = epp.tile([P, NSL], f32, name=f"t3_{j}_{g}",
                                  tag="t3")
                    nc.vector.tensor_sub(t3[:], st_sl[:], t2[:])
                    hsl = h_t[g][:, jsl]
                    if has_logstep:
                        nc.vector.tensor_mul(t3[:], t3[:], alpha_t[:, jsl])
                        nc.vector.tensor_add(out=hsl, in0=t2[:], in1=t3[:])
                    else:
                        # h = t2 + ALPHA0 * t3, written to bf16 h_t
                        nc.vector.scalar_tensor_tensor(
                            hsl, t3[:], ALPHA0, t2[:],
                            op0=OP.mult, op1=OP.add)

                    nc.vector.bn_stats(out=stats_t[g][:, j, :], in_=hsl)

                    if j == NJ - 1:
                        # layernorm + output for this group, overlapping
                        # the remaining groups' matmuls
                        mv = normp.tile([P, 2], f32, name=f"mv_{g}",
                                        tag="mv")
                        nc.vector.bn_aggr(out=mv[:], in_=stats_t[g][:])
                        rstd = normp.tile([P, 1], f32, name=f"rstd_{g}",
                                          tag="rstd")
                        nc.scalar.activation(rstd[:], mv[:, 1:2], AF.Sqrt,
                                             bias=eps_t[:])
                        nc.vector.reciprocal(rstd[:], rstd[:])
                        ot = outp.tile([P, H], bft, name=f"ot_{g}",
                                       tag="ot")
                        HH = H // 4
                        for part in range(4):
                            hs = slice(part * HH, (part + 1) * HH)
                            nc.vector.tensor_scalar(
                                ot[:, hs], h_t[g][:, hs],
                                mv[:, 0:1], rstd[:],
                                op0=OP.subtract, op1=OP.mult)
                            if has_gamma:
                                nc.vector.tensor_mul(ot[:, hs], ot[:, hs],
                                                     vt["gammab"][:, hs])
                            if has_beta:
                                nc.vector.tensor_add(ot[:, hs], ot[:, hs],
                                                     vt["betab"][:, hs])
                            nc.sync.dma_start(
                                out=out[g * P:(g + 1) * P, hs],
                                in_=ot[:, hs])

    nc.compile()
    return nc


def _get_compiled(flags):
    if flags not in _compiled:
        _compiled[flags] = _build(flags)
    return _compiled[flags]


def kernel(x_t, state, Wc, Uc, bc, Wg, Ug, bg, log_step, gamma, beta):
    global LAST_RESULTS
    from concourse import bass_utils

    x_t = np.asarray(x_t, np.float32)
    state = np.asarray(state, np.float32)
    Wc = np.asarray(Wc, np.float32)
    Uc = np.asarray(Uc, np.float32)
    Wg = np.asarray(Wg, np.float32)
    Ug = np.asarray(Ug, np.float32)
    bc = np.asarray(bc, np.float32)
    bg = np.asarray(bg, np.float32)
    log_step = np.asarray(log_step, np.float32)
    gamma = np.asarray(gamma, np.float32)
    beta = np.asarray(beta, np.float32)

    # fold the recurrent weights and pre-tile for the device:
    # [j, p, k, n] = W[k*128+p, j*W_SL+n]
    def wtile(w, dt, scale, nj, nsl):
        return np.ascontiguousarray(
            (w * scale).astype(dt).reshape(KT, P, nj, nsl)
            .transpose(2, 1, 0, 3))

    Wcs_f = Wc[IN:] + Uc
    Wgs_f = Wg[IN:] + Ug

    def wtile_k(w, dt, scale, nk):
        # [j, p, k, n] tiling for a weight slab with nk k-tiles
        return np.ascontiguousarray(
            (w * scale).astype(dt).reshape(nk, P, NJG, NGL)
            .transpose(2, 1, 0, 3))

    w_maps = {
        # cand x part rides the gate's fp8 activations; the first FSK
        # k-tiles of its state part ride the gate's fp8 state activations;
        # the remaining state k-tiles are bf16 with weights pre-scaled to
        # the fp8 PSUM scale (SX*SW = 2^12, exact in bf16) so one PSUM
        # stream and one descale serve the whole path.
        "wcx": wtile(Wc[:IN], e4m3, SW, NJG, NGL),
        "wcs8": wtile_k(Wcs_f[:FSK * P], e4m3, SW, FSK),
        "wcs": wtile_k(Wcs_f[FSK * P:], bf16, SX * SW, KTB),
        "wgx": wtile(Wg[:IN], e4m3, SW, NJG, NGL),
        "wgs": wtile(Wgs_f, e4m3, SW, NJG, NGL),
    }
    flags = (bool(bc.any()), bool(bg.any()),
             bool((gamma != 1.0).any()), bool(beta.any()),
             bool(log_step.any()))
    vec_maps = {}
    if flags[4]:
        vec_maps["logb"] = np.ascontiguousarray(
            np.broadcast_to(log_step.reshape(1, H), (P, H)))
    if flags[0]:
        vec_maps["bcb"] = np.ascontiguousarray(
            np.broadcast_to(bc.reshape(1, H), (P, H)))
    if flags[1]:
        vec_maps["bgb"] = np.ascontiguousarray(
            np.broadcast_to(bg.reshape(1, H), (P, H)))
    if flags[2]:
        vec_maps["gammab"] = np.ascontiguousarray(
            np.broadcast_to(gamma.reshape(1, H), (P, H)))
    if flags[3]:
        vec_maps["betab"] = np.ascontiguousarray(
            np.broadcast_to(beta.reshape(1, H), (P, H)))

    nc = _get_compiled(flags)

    # per-core activation shards, pre-tiled: [g, p, k, m] = x[g*128+m, k*128+p]
    def atile(a, dt, scale, nk=KT):
        return np.ascontiguousarray(
            (a * scale).astype(dt).reshape(G, P, nk, P).transpose(0, 3, 2, 1))

    in_maps = []
    for c in range(NCORES):
        rows = slice(c * BC, (c + 1) * BC)
        m = {
            "sb4": atile(state[rows, FSK * P:], bf16, 1.0, KTB),
            "xq4": atile(x_t[rows], e4m3, SX),
            "sq4": atile(state[rows], e4m3, SX),
            "stb": np.ascontiguousarray(state[rows].astype(bf16)),
        }
        m.update(w_maps)
        m.update(vec_maps)
        in_maps.append(m)

    trace_kwargs = {}
    if TRACE:
        trace_kwargs["trace_cores"] = list(range(NCORES))
    res = bass_utils.run_bass_kernel_spmd(
        nc, in_maps, core_ids=list(range(NCORES)), trace=TRACE,
        **trace_kwargs)
    LAST_RESULTS = res
    return np.concatenate(
        [res.results[c]["out"] for c in range(NCORES)],
        axis=0).astype(np.float32)



# revision 34
# speedup vs baseline: 1.0079x; 1.0079x over previous
"""Trainium2 Bass kernel for nn_BrainRegion (liquid-gated recurrent cell).

Computes, for full inputs (B=8192, IN=H=2048):
    xin  = concat([x_t, state], -1)
    cand = tanh(xin @ Wc + state @ Uc + bc)
    gate = sigmoid(xin @ Wg + state @ Ug + bg)
    alpha = exp(-1/exp(log_step))
    h    = alpha * state + (1 - alpha) * gate * cand
    out  = layernorm(h) * gamma + beta

Strategy: data-parallel over batch across 8 NeuronCores (1024 rows/core),
weights replicated.  Algebraic fold: xin@Wc + state@Uc == x_t@Wc[:IN] +
state@(Wc[IN:] + Uc), which removes one third of the FLOPs.  Mixed
precision: the sigmoid (gate) path runs entirely in fp8 e4m3 with
DoubleRow perf mode (2x tensor throughput; sigmoid' <= 0.25 compresses
the quantization error).  The tanh (cand) path splits: its x_t part is
fp8 DoubleRow (reusing the gate's quantized activations), while its
state part stays bf16 -- the folded state weights (Wc[IN:]+Uc) carry 3x
the variance of Wc[:IN], so they dominate the quantization error and
are kept in high precision.  The bf16 weights are pre-scaled by
4096 == SX*SW (an exact power of two) so both parts accumulate into a
single PSUM stream and one epilogue descale serves the whole path.
PSUM accumulates in fp32; the elementwise epilogue + layernorm run
on-device in fp32; h/state/output in bf16.
"""

import sys

if "/opt/trn_rl_repo" not in sys.path:
    sys.path.insert(0, "/opt/trn_rl_repo")

import numpy as np
import ml_dtypes

B, IN, H = 8192, 2048, 2048
NCORES = 8
BC = B // NCORES      # rows per core (1024)
P = 128               # partitions
G = BC // P           # batch groups per core (8)
NJ = 8                # H slices for cand/epilogue
NSL = H // NJ         # slice width (256)
NJG = 4               # H slices for the fp8 gate matmuls
NGL = H // NJG        # gate slice width (512)
KT = H // P           # k-tiles per matrix (16)
KP = KT // 2          # fp8 DoubleRow k-pairs (8)
FSK = 2               # cand-s k-tiles computed in fp8 (error budget cap)
KTB = KT - FSK        # cand-s k-tiles in bf16 (14)
EPS = 1e-5

bf16 = ml_dtypes.bfloat16
e4m3 = ml_dtypes.float8_e4m3
SX = 16.0             # gate activation quant scale
SW = 256.0            # gate weight quant scale
DESCALE = 1.0 / (SX * SW)

# Set by test.py to collect a hardware profile.
TRACE = False
LAST_RESULTS = None

_compiled = {}


ALPHA0 = float(np.exp(-1.0))  # alpha when log_step == 0


def _build(flags):
    """Trace + compile the SPMD device program. flags = (has_bc, has_bg,
    has_gamma, has_beta, has_logstep) selects optional elementwise
    passes."""
    from contextlib import ExitStack

    import concourse.bass as bass
    import concourse.tile as tile
    from concourse import bacc, mybir

    has_bc, has_bg, has_gamma, has_beta, has_logstep = flags
    f32 = mybir.dt.float32
    bft = mybir.dt.bfloat16
    f8 = mybir.dt.float8e4
    DR = mybir.MatmulPerfMode.DoubleRow
    AF = mybir.ActivationFunctionType
    OP = mybir.AluOpType

    nc = bacc.Bacc("TRN2", target_bir_lowering=False, debug=False,
                   num_devices=NCORES)

    # DRAM I/O. Activation tensors are pre-arranged on host so every DMA
    # below is contiguous:
    #   sb4:     [G, P, KTB, P]  bf16, [g,p,k,m] = s[g*128+m, (k+FSK)*128+p]
    #   xq4/sq4: [G, P, KT, P]   fp8 (x*SX), [g,p,k,m] = x[g*128+m, k*128+p]
    #   wcx:     [NJG, P, KT, NGL] fp8 (W*SW), [j,p,k,n] = W[k*128+p, j*NGL+n]
    #   wcs8:    [NJG, P, FSK, NGL] fp8 (W*SW), first FSK k-tiles of Wcs
    #   wcs:     [NJG, P, KTB, NGL] bf16 (W*SX*SW), remaining k-tiles
    #   wg*:     [NJG, P, KT, NGL] fp8 (W*SW), same arrangement
    sb4 = nc.dram_tensor("sb4", [G, P, KTB, P], bft,
                         kind="ExternalInput").ap()
    xq4 = nc.dram_tensor("xq4", [G, P, KT, P], f8, kind="ExternalInput").ap()
    sq4 = nc.dram_tensor("sq4", [G, P, KT, P], f8, kind="ExternalInput").ap()
    stb = nc.dram_tensor("stb", [BC, H], bft, kind="ExternalInput").ap()
    wcx = nc.dram_tensor("wcx", [NJG, P, KT, NGL], f8,
                         kind="ExternalInput").ap()
    wcs8 = nc.dram_tensor("wcs8", [NJG, P, FSK, NGL], f8,
                          kind="ExternalInput").ap()
    wcs = nc.dram_tensor("wcs", [NJG, P, KTB, NGL], bft,
                         kind="ExternalInput").ap()
    wgx = nc.dram_tensor("wgx", [NJG, P, KT, NGL], f8,
                         kind="ExternalInput").ap()
    wgs = nc.dram_tensor("wgs", [NJG, P, KT, NGL], f8,
                         kind="ExternalInput").ap()
    if has_logstep:
        logb = nc.dram_tensor("logb", [P, H], f32,
                              kind="ExternalInput").ap()
    vecs = {}
    for name, used in (("bcb", has_bc), ("bgb", has_bg),
                       ("gammab", has_gamma), ("betab", has_beta)):
        if used:
            vecs[name] = nc.dram_tensor(name, [P, H], f32,
                                        kind="ExternalInput").ap()
    out = nc.dram_tensor("out", [BC, H], bft, kind="ExternalOutput").ap()

    with tile.TileContext(nc) as tc, ExitStack() as ctx:
        singles = ctx.enter_context(tc.tile_pool(name="singles", bufs=1))
        gactp = ctx.enter_context(tc.tile_pool(name="gactp", bufs=1))
        cactp = ctx.enter_context(tc.tile_pool(name="cactp", bufs=2))
        wcp = ctx.enter_context(tc.tile_pool(name="wcp", bufs=2))
        wgp = ctx.enter_context(tc.tile_pool(name="wgp", bufs=2))
        psgp = ctx.enter_context(tc.tile_pool(name="psgp", bufs=2,
                                              space="PSUM"))
        pscp = ctx.enter_context(tc.tile_pool(name="pscp", bufs=3,
                                              space="PSUM"))
        epp = ctx.enter_context(tc.tile_pool(name="epp", bufs=2))
        stp = ctx.enter_context(tc.tile_pool(name="stp", bufs=3))
        hp = ctx.enter_context(tc.tile_pool(name="hp", bufs=1))
        statp = ctx.enter_context(tc.tile_pool(name="statp", bufs=1))
        normp = ctx.enter_context(tc.tile_pool(name="normp", bufs=4))
        outp = ctx.enter_context(tc.tile_pool(name="outp", bufs=2))

        # ---- gate fp8 activations: resident for the whole kernel.
        # DMA'd lazily inside the first jg sweep so the first weight
        # slices aren't stuck behind 4 MB of activation DMA.
        xq_t = [gactp.tile([P, KT, P], f8, name=f"xq_g{g}", tag=f"xq{g}")
                for g in range(G)]
        sq_t = [gactp.tile([P, KT, P], f8, name=f"sq_g{g}", tag=f"sq{g}")
                for g in range(G)]

        # ---- constants: alpha = exp(-exp(-log_step)), broadcast [P, H].
        # When log_step == 0 (has_logstep False) alpha is the compile-time
        # scalar ALPHA0 and no tile is needed.
        if has_logstep:
            alpha_t = singles.tile([P, H], f32, name="alpha_t")
            nc.sync.dma_start(out=alpha_t[:], in_=logb[:])
            nc.scalar.activation(alpha_t[:], alpha_t[:], AF.Exp, scale=-1.0)
            nc.scalar.activation(alpha_t[:], alpha_t[:], AF.Exp, scale=-1.0)
        eps_t = singles.tile([P, 1], f32, name="eps_t")
        nc.vector.memset(eps_t[:], EPS)
        vt = {}
        for name in vecs:
            vt[name] = singles.tile([P, H], f32, name=name + "_t")
            nc.sync.dma_start(out=vt[name][:], in_=vecs[name][:])

        # ---- per-group h accumulator (bf16) and layernorm stats ----
        h_t = [hp.tile([P, H], bft, name=f"h_g{g}", tag=f"h{g}")
               for g in range(G)]
        stats_t = [statp.tile([P, NJ, 6], f32, name=f"stats_g{g}",
                              tag=f"st{g}")
                   for g in range(G)]

        # ---- PE warm-up: the tensor clock ramps over ~3us of continuous
        # work (first matmuls run 2-3x slow).  Burn the ramp on dummy
        # matmuls over a memset tile while the first weight/activation
        # DMAs are still in flight.  The PSUM target cycles through the
        # cand pool so no extra bank is pinned.
        warm_src = singles.tile([P, NGL], bft, name="warm_src")
        nc.vector.memset(warm_src[:], 0.0)
        warm_ps = pscp.tile([P, NGL], f32, name="warm_ps", tag="pc")
        for i in range(9):
            nc.tensor.matmul(warm_ps[:], warm_src[:, :P], warm_src[:],
                             start=True, stop=True)

        # ---- main loops: jg = gate H slice (2 cand slices), g = batch ----
        prefetched = {}
        for jg in range(NJG):
            wgx_t = wgp.tile([P, KT, NGL], f8, name=f"wgx_{jg}", tag="wgx")
            wgs_t = wgp.tile([P, KT, NGL], f8, name=f"wgs_{jg}", tag="wgs")
            wcx_t = wcp.tile([P, KT, NGL], f8, name=f"wcx_{jg}", tag="wcx")
            wcs8_t = wcp.tile([P, FSK, NGL], f8, name=f"wcs8_{jg}",
                              tag="wcs8")
            wcs_t = wcp.tile([P, KTB, NGL], bft, name=f"wcs_{jg}", tag="wcs")
            if jg == 0:
                # First iteration: every matmul stream is on the critical
                # path.  Chunk the weight tiles k-wise so the 14 streams
                # land on distinct DMA queues (a queue moves ~8KB/us; a
                # whole tile on one queue costs 16us); issue in consumption
                # order: gate-x, gate-s, cand-x, cand-s.
                KH = KT // 2
                nc.sync.dma_start(out=wgx_t[:, :KH, :],
                                  in_=wgx[jg, :, :KH, :])
                nc.sync.dma_start(out=xq_t[0][:], in_=xq4[0])
                nc.sync.dma_start(out=sq_t[0][:], in_=sq4[0])
                nc.sync.dma_start(out=wgx_t[:, KH:, :],
                                  in_=wgx[jg, :, KH:, :])
                # wcs (1.75MB bf16) is the largest first-iteration tile:
                # 4-way chunks, issued early so every chunk lands before
                # the cand-s stream (~19us in)
                for ck, ce in ((0, 4), (4, 8), (8, 11), (11, KTB)):
                    nc.sync.dma_start(out=wcs_t[:, ck:ce, :],
                                      in_=wcs[jg, :, ck:ce, :])
                for c in range(2):
                    nc.sync.dma_start(out=wgs_t[:, c * KH:(c + 1) * KH, :],
                                      in_=wgs[jg, :, c * KH:(c + 1) * KH, :])
                for c in range(2):
                    nc.sync.dma_start(out=wcx_t[:, c * KH:(c + 1) * KH, :],
                                      in_=wcx[jg, :, c * KH:(c + 1) * KH, :])
                nc.sync.dma_start(out=wcs8_t[:], in_=wcs8[jg])
                sb0 = cactp.tile([P, KTB, P], bft, name="sb_0_0", tag="sb")
                KBH = KTB // 2
                nc.sync.dma_start(out=sb0[:, :KBH, :],
                                  in_=sb4[0, :, :KBH, :])
                nc.sync.dma_start(out=sb0[:, KBH:, :],
                                  in_=sb4[0, :, KBH:, :])
                prefetched[(0, 0)] = sb0
            else:
                nc.sync.dma_start(out=wgx_t[:], in_=wgx[jg])
                nc.sync.dma_start(out=wgs_t[:], in_=wgs[jg])
                nc.sync.dma_start(out=wcx_t[:], in_=wcx[jg])
                nc.sync.dma_start(out=wcs8_t[:], in_=wcs8[jg])
                nc.sync.dma_start(out=wcs_t[:], in_=wcs[jg])

            for g in range(G):
                if jg == 0 and g > 0:
                    nc.sync.dma_start(out=xq_t[g][:], in_=xq4[g])
                    nc.sync.dma_start(out=sq_t[g][:], in_=sq4[g])
                # stream the cand bf16 state activations for this (jg, g)
                if (jg, g) in prefetched:
                    sb_t = prefetched.pop((jg, g))
                else:
                    sb_t = cactp.tile([P, KTB, P], bft, name=f"sb_{jg}_{g}",
                                      tag="sb")
                    nc.sync.dma_start(out=sb_t[:], in_=sb4[g])

                # gate: fp8 DoubleRow matmuls, 512-wide moving stream
                pg = psgp.tile([P, NGL], f32, name=f"pg_{jg}_{g}", tag="pg")
                for kp in range(KP):
                    ks = slice(2 * kp, 2 * kp + 2)
                    nc.tensor.matmul(pg[:], xq_t[g][:, ks, :],
                                     wgx_t[:, ks, :],
                                     start=(kp == 0), stop=False,
                                     perf_mode=DR)
                for kp in range(KP):
                    ks = slice(2 * kp, 2 * kp + 2)
                    nc.tensor.matmul(pg[:], sq_t[g][:, ks, :],
                                     wgs_t[:, ks, :],
                                     start=False, stop=(kp == KP - 1),
                                     perf_mode=DR)

                # cand: x part fp8 DoubleRow, first FSK state k-tiles fp8
                # DoubleRow (via the resident gate activations), remaining
                # state k-tiles bf16 (weights pre-scaled by SX*SW so all
                # parts land at the same PSUM scale)
                pc = pscp.tile([P, NGL], f32, name=f"pc_{jg}_{g}",
                               tag="pc")
                for kp in range(KP):
                    ks = slice(2 * kp, 2 * kp + 2)
                    nc.tensor.matmul(pc[:], xq_t[g][:, ks, :],
                                     wcx_t[:, ks, :],
                                     start=(kp == 0), stop=False,
                                     perf_mode=DR)
                for kp in range(FSK // 2):
                    ks = slice(2 * kp, 2 * kp + 2)
                    nc.tensor.matmul(pc[:], sq_t[g][:, ks, :],
                                     wcs8_t[:, ks, :],
                                     start=False, stop=False,
                                     perf_mode=DR)
                for k in range(KTB):
                    nc.tensor.matmul(pc[:], sb_t[:, k, :],
                                     wcs_t[:, k, :],
                                     start=False, stop=(k == KTB - 1))

                for j in (2 * jg, 2 * jg + 1):
                    jsl = slice(j * NSL, (j + 1) * NSL)
                    off = (j - 2 * jg) * NSL
                    pcs = pc[:, off:off + NSL]
                    pgs = pg[:, off:off + NSL]

                    # epilogue for this (g, j) slice
                    sc = epp.tile([P, NSL], f32, name=f"sc_{j}_{g}",
                                  tag="sc")
                    sg = epp.tile([P, NSL], f32, name=f"sg_{j}_{g}",
                                  tag="sg")
                    if has_bc:
                        nc.vector.scalar_tensor_tensor(
                            sc[:], pcs, DESCALE, vt["bcb"][:, jsl],
                            op0=OP.mult, op1=OP.add)
                        nc.scalar.activation(sc[:], sc[:], AF.Tanh)
                    else:
                        nc.scalar.activation(sc[:], pcs, AF.Tanh,
                                             scale=DESCALE)
                    if has_bg:
                        nc.vector.scalar_tensor_tensor(
                            sg[:], pgs, DESCALE, vt["bgb"][:, jsl],
                            op0=OP.mult, op1=OP.add)
                        nc.scalar.activation(sg[:], sg[:], AF.Sigmoid)
                    else:
                        nc.scalar.activation(sg[:], pgs, AF.Sigmoid,
                                             scale=DESCALE)

                    st_sl = stp.tile([P, NSL], bft, name=f"stsl_{j}_{g}",
                                     tag="stsl")
                    nc.sync.dma_start(
                        out=st_sl[:],
                        in_=stb[g * P:(g + 1) * P, jsl])

                    # h = gc + alpha*(state - gc), with gc = gate*cand
                    t2 = epp.tile([P, NSL], f32, name=f"t2_{j}_{g}",
                                  tag="t2")
                    nc.vector.tensor_mul(t2[:], sc[:], sg[:])   # gate*cand
                    t3 = epp.tile([P, NSL], f32, name=f"t3_{j}_{g}",
                                  tag="t3")
                    nc.vector.tensor_sub(t3[:], st_sl[:], t2[:])
                    hsl = h_t[g][:, jsl]
                    if has_logstep:
                        nc.vector.tensor_mul(t3[:], t3[:], alpha_t[:, jsl])
                        nc.vector.tensor_add(out=hsl, in0=t2[:], in1=t3[:])
                    else:
                        # h = t2 + ALPHA0 * t3, written to bf16 h_t
                        nc.vector.scalar_tensor_tensor(
                            hsl, t3[:], ALPHA0, t2[:],
                            op0=OP.mult, op1=OP.add)

                    nc.vector.bn_stats(out=stats_t[g][:, j, :], in_=hsl)

                    if j == NJ - 1:
                        # layernorm + output for this group, overlapping
                        # the remaining groups' matmuls
                        mv = normp.tile([P, 2], f32, name=f"mv_{g}",
                                        tag="mv")
                        nc.vector.bn_aggr(out=mv[:], in_=stats_t[g][:])
                        rstd = normp.tile([P, 1], f32, name=f"rstd_{g}",
                                          tag="rstd")
                        nc.scalar.activation(rstd[:], mv[:, 1:2], AF.Sqrt,
                                             bias=eps_t[:])
                        nc.vector.reciprocal(rstd[:], rstd[:])
                        ot = outp.tile([P, H], bft, name=f"ot_{g}",
                                       tag="ot")
                        HH = H // 4
                        for part in range(4):
                            hs = slice(part * HH, (part + 1) * HH)
                            nc.vector.tensor_scalar(
                                ot[:, hs], h_t[g][:, hs],
                                mv[:, 0:1], rstd[:],
                                op0=OP.subtract, op1=OP.mult)
                            if has_gamma:
                                nc.vector.tensor_mul(ot[:, hs], ot[:, hs],
                                                     vt["gammab"][:, hs])
                            if has_beta:
                                nc.vector.tensor_add(ot[:, hs], ot[:, hs],
                                                     vt["betab"][:, hs])
                            nc.sync.dma_start(
                                out=out[g * P:(g + 1) * P, hs],
                                in_=ot[:, hs])

    nc.compile()
    return nc


def _get_compiled(flags):
    if flags not in _compiled:
        _compiled[flags] = _build(flags)
    return _compiled[flags]


def kernel(x_t, state, Wc, Uc, bc, Wg, Ug, bg, log_step, gamma, beta):
    global LAST_RESULTS
    from concourse import bass_utils

    x_t = np.asarray(x_t, np.float32)
    state = np.asarray(state, np.float32)
    Wc = np.asarray(Wc, np.float32)
    Uc = np.asarray(Uc, np.float32)
    Wg = np.asarray(Wg, np.float32)
    Ug = np.asarray(Ug, np.float32)
    bc = np.asarray(bc, np.float32)
    bg = np.asarray(bg, np.float32)
    log_step = np.asarray(log_step, np.float32)
    gamma = np.asarray(gamma, np.float32)
    beta = np.asarray(beta, np.float32)

    # fold the recurrent weights and pre-tile for the device:
    # [j, p, k, n] = W[k*128+p, j*W_SL+n]
    def wtile(w, dt, scale, nj, nsl):
        return np.ascontiguousarray(
            (w * scale).astype(dt).reshape(KT, P, nj, nsl)
            .transpose(2, 1, 0, 3))

    Wcs_f = Wc[IN:] + Uc
    Wgs_f = Wg[IN:] + Ug

    def wtile_k(w, dt, scale, nk):
        # [j, p, k, n] tiling for a weight slab with nk k-tiles
        return np.ascontiguousarray(
            (w * scale).astype(dt).reshape(nk, P, NJG, NGL)
            .transpose(2, 1, 0, 3))

    w_maps = {
        # cand x part rides the gate's fp8 activations; the first FSK
        # k-tiles of its state part ride the gate's fp8 state activations;
        # the remaining state k-tiles are bf16 with weights pre-scaled to
        # the fp8 PSUM scale (SX*SW = 2^12, exact in bf16) so one PSUM
        # stream and one descale serve the whole path.
        "wcx": wtile(Wc[:IN], e4m3, SW, NJG, NGL),
        "wcs8": wtile_k(Wcs_f[:FSK * P], e4m3, SW, FSK),
        "wcs": wtile_k(Wcs_f[FSK * P:], bf16, SX * SW, KTB),
        "wgx": wtile(Wg[:IN], e4m3, SW, NJG, NGL),
        "wgs": wtile(Wgs_f, e4m3, SW, NJG, NGL),
    }
    flags = (bool(bc.any()), bool(bg.any()),
             bool((gamma != 1.0).any()), bool(beta.any()),
             bool(log_step.any()))
    vec_maps = {}
    if flags[4]:
        vec_maps["logb"] = np.ascontiguousarray(
            np.broadcast_to(log_step.reshape(1, H), (P, H)))
    if flags[0]:
        vec_maps["bcb"] = np.ascontiguousarray(
            np.broadcast_to(bc.reshape(1, H), (P, H)))
    if flags[1]:
        vec_maps["bgb"] = np.ascontiguousarray(
            np.broadcast_to(bg.reshape(1, H), (P, H)))
    if flags[2]:
        vec_maps["gammab"] = np.ascontiguousarray(
            np.broadcast_to(gamma.reshape(1, H), (P, H)))
    if flags[3]:
        vec_maps["betab"] = np.ascontiguousarray(
            np.broadcast_to(beta.reshape(1, H), (P, H)))

    nc = _get_compiled(flags)

    # per-core activation shards, pre-tiled: [g, p, k, m] = x[g*128+m, k*128+p]
    def atile(a, dt, scale, nk=KT):
        return np.ascontiguousarray(
            (a * scale).astype(dt).reshape(G, P, nk, P).transpose(0, 3, 2, 1))

    in_maps = []
    for c in range(NCORES):
        rows = slice(c * BC, (c + 1) * BC)
        m = {
            "sb4": atile(state[rows, FSK * P:], bf16, 1.0, KTB),
            "xq4": atile(x_t[rows], e4m3, SX),
            "sq4": atile(state[rows], e4m3, SX),
            "stb": np.ascontiguousarray(state[rows].astype(bf16)),
        }
        m.update(w_maps)
        m.update(vec_maps)
        in_maps.append(m)

    trace_kwargs = {}
    if TRACE:
        trace_kwargs["trace_cores"] = list(range(NCORES))
    res = bass_utils.run_bass_kernel_spmd(
        nc, in_maps, core_ids=list(range(NCORES)), trace=TRACE,
        **trace_kwargs)
    LAST_RESULTS = res
    return np.concatenate(
        [res.results[c]["out"] for c in range(NCORES)],
        axis=0).astype(np.float32)

